# revision 4
# baseline (speedup 1.0000x reference)
"""Trainium2 Bass kernel: ClusterActivation (nearest-centroid routing +
per-row normalization + per-cluster activation).

Data-parallel over 8 NeuronCores: x sharded along rows (padded to a
multiple of 128*G), centroids replicated. Per core, rows are processed
in [128, 512] tiles, G tiles per group:

  - nearest centroid: PE transposes the x tile (f32 exact), ScalarE
    copies PSUM->SBUF, then f32 matmuls against centroid chunks
    accumulate [x @ c^T | row_sum] in PSUM (a 5th ones-column gives the
    row sum for free). A DVE scalar_tensor_tensor adds -|c_j|^2/2;
    argmax via reduce + is_equal gives one-hot masks.
  - stats: sum(x^2) via a DVE scalar_tensor_tensor with accum_out;
    mean from the PE ones-column; inv_std = rsqrt via Newton on DVE.
  - activations: two ScalarE passes from ONE LUT set
    (gelu_apprx_tanh_and_others):
      pass G: Gelu_apprx_tanh(x*sg + bg) covers gelu rows (scale α),
        relu rows (scale KF*α; gelu(KF z)/KF == relu(z) for large KF —
        LUT tail verified exact on HW), and sigmoid rows (scale 0,
        bias b* with gelu(b*)=0.5).
      pass T: Tanh(x*st + bt) covers tanh rows (a=1) and sigmoid rows
        (a=0.5; sigmoid(z) = 0.5 tanh(z/2) + 0.5).
    GpSimd computes c2 = tanh_out * a_t; DVE combines
    out = pg * W + c2 in one scalar_tensor_tensor.
  - per-row coefficient algebra is batched across the G tiles of a
    group ([128, G] ops), amortizing DVE instruction overhead.
"""

import numpy as np
from contextlib import ExitStack

import concourse.bacc as bacc
import concourse.mybir as mybir
from concourse.tile import TileContext
from concourse import bass_utils

F32 = mybir.dt.float32
AF = mybir.ActivationFunctionType
OP = mybir.AluOpType
AX = mybir.AxisListType

N_TOTAL, D, C = 100000, 512, 4
N_CORES = 8
P = 128
G = 7  # tiles per group
ROWS = 12544  # padded rows per core: 98 tiles = 14 groups of 7
ROWS_REAL = N_TOTAL // N_CORES  # 12500
KCHUNKS = D // P  # 4
EPS = 1e-5
KF = 1024.0  # relu-via-gelu factor
INV_N = 1.0 / D
INV_NM1 = 1.0 / (D - 1.0)
BIAS_N = D * INV_NM1  # 512/511


def _solve_bstar():
    """b with gelu_tanh(b) == 0.5."""
    def g(z):
        return 0.5 * z * (1 + np.tanh(np.sqrt(2 / np.pi) * (z + 0.044715 * z**3)))

    lo, hi = 0.1, 1.5
    for _ in range(200):
        mid = 0.5 * (lo + hi)
        if g(mid) < 0.5:
            lo = mid
        else:
            hi = mid
    return float(0.5 * (lo + hi))


B_STAR = _solve_bstar()


def build_program(rows=ROWS, g_tiles=G):
    assert rows % (P * g_tiles) == 0
    n_groups = rows // (P * g_tiles)
    nc = bacc.Bacc("TRN2", target_bir_lowering=False, debug=False)
    x = nc.dram_tensor("x", [rows, D], F32, kind="ExternalInput").ap()
    # ct[d, k*(C+1) + j] = centroids[j, k*128+d] for j<C; ones for j=C
    ct = nc.dram_tensor("ct", [P, KCHUNKS * (C + 1)], F32, kind="ExternalInput").ap()
    # negn_rep[p, j] = -|c_j|^2/2 for j<C, 0 for j=C (replicated rows)
    negn = nc.dram_tensor("negn", [P, C + 1], F32, kind="ExternalInput").ap()
    ident = nc.dram_tensor("ident", [P, P], F32, kind="ExternalInput").ap()
    out = nc.dram_tensor("out", [rows, D], F32, kind="ExternalOutput").ap()

    with ExitStack() as ctx:
        tc = ctx.enter_context(TileContext(nc))
        cpool = ctx.enter_context(tc.tile_pool(name="const", bufs=1))
        xpool = ctx.enter_context(tc.tile_pool(name="xin", bufs=2))
        xtpool = ctx.enter_context(tc.tile_pool(name="xt", bufs=3))
        scpool = ctx.enter_context(tc.tile_pool(name="scratch", bufs=2))
        apool = ctx.enter_context(tc.tile_pool(name="acts", bufs=3))
        opool = ctx.enter_context(tc.tile_pool(name="outs", bufs=2))
        spool = ctx.enter_context(tc.tile_pool(name="small", bufs=2))
        ptpool = ctx.enter_context(tc.tile_pool(name="pxt", bufs=3, space="PSUM"))
        pspool = ctx.enter_context(tc.tile_pool(name="psc", bufs=3, space="PSUM"))

        ct_sb = cpool.tile([P, KCHUNKS, C + 1], F32)
        nc.sync.dma_start(ct_sb[:], ct.rearrange("d (k j) -> d k j", k=KCHUNKS))
        id_sb = cpool.tile([P, P], F32)
        nc.sync.dma_start(id_sb[:], ident)
        negn_sb = cpool.tile([P, C + 1], F32)
        nc.sync.dma_start(negn_sb[:], negn)

        xg = x.rearrange("(n g p) d -> n g p d", g=g_tiles, p=P)
        og = out.rearrange("(n g p) d -> n g p d", g=g_tiles, p=P)

        for n in range(n_groups):
            xt = xpool.tile([P, g_tiles, D], F32, tag="x")
            nc.sync.dma_start(xt[:], xg[n].rearrange("g p d -> p g d"))

            # group arrays
            adj = spool.tile([P, g_tiles, C + 1], F32, tag="adj")
            ssq = spool.tile([P, g_tiles], F32, tag="ssq")
            smax = spool.tile([P, g_tiles], F32, tag="smax")
            masks = spool.tile([P, g_tiles, C], F32, tag="masks")

            for g in range(g_tiles):
                xs = xt[:, g, :]
                # transpose x tile -> PSUM -> SBUF
                pxt = ptpool.tile([P, KCHUNKS, P], F32, tag="pxt")
                for k in range(KCHUNKS):
                    nc.tensor.transpose(
                        pxt[:, k, :], xs[:, k * P : (k + 1) * P], id_sb[:]
                    )
                xtsb = xtpool.tile([P, KCHUNKS, P], F32, tag="xtsb")
                nc.scalar.copy(xtsb[:], pxt[:])
                # scores + row-sum
                psc = pspool.tile([P, C + 1], F32, tag="psc")
                for k in range(KCHUNKS):
                    nc.tensor.matmul(
                        psc[:],
                        lhsT=xtsb[:, k, :],
                        rhs=ct_sb[:, k, :],
                        start=(k == 0),
                        stop=(k == KCHUNKS - 1),
                    )
                # adj = psc + [-|c|^2/2, 0]
                nc.vector.scalar_tensor_tensor(
                    adj[:, g, :], psc[:], 1.0, negn_sb[:], OP.mult, OP.add
                )
                # sum(x^2) with accumulate
                sq = scpool.tile([P, D], F32, tag="sq")
                nc.vector.scalar_tensor_tensor(
                    sq[:], xs, 1.0, xs, OP.bypass, OP.mult,
                    accum_out=ssq[:, g : g + 1],
                )
                nc.vector.tensor_reduce(
                    smax[:, g : g + 1], adj[:, g, 0:C], axis=AX.X, op=OP.max
                )
                nc.vector.tensor_scalar(
                    masks[:, g, :], adj[:, g, 0:C], smax[:, g : g + 1], None,
                    OP.is_equal,
                )

            # ---- batched per-row coefficient algebra ([P, G] ops) ----
            m_r = masks[:, :, 0]
            m_t = masks[:, :, 1]
            m_g = masks[:, :, 2]
            m_s = masks[:, :, 3]
            mean = spool.tile([P, g_tiles], F32, tag="mean")
            nc.vector.tensor_scalar(mean[:], adj[:, :, C], INV_N, None, OP.mult)
            msq = spool.tile([P, g_tiles], F32, tag="msq")
            nc.vector.tensor_tensor(msq[:], mean[:], mean[:], OP.mult)
            u = spool.tile([P, g_tiles], F32, tag="u")
            nc.vector.tensor_scalar(u[:], msq[:], BIAS_N, -EPS, OP.mult, OP.add)
            vv = spool.tile([P, g_tiles], F32, tag="vv")
            nc.vector.scalar_tensor_tensor(
                vv[:], ssq[:], INV_NM1, u[:], OP.mult, OP.subtract
            )
            al = spool.tile([P, g_tiles], F32, tag="seed")
            nc.vector.tensor_scalar(al[:], vv[:], -0.5, 1.5, OP.mult, OP.add)
            for it in range(2):
                t2 = spool.tile([P, g_tiles], F32, tag=f"nw{it}a")
                nc.vector.tensor_tensor(t2[:], al[:], al[:], OP.mult)
                t3 = spool.tile([P, g_tiles], F32, tag=f"nw{it}b")
                nc.vector.tensor_tensor(t3[:], t2[:], vv[:], OP.mult)
                w = spool.tile([P, g_tiles], F32, tag=f"nw{it}c")
                nc.vector.tensor_scalar(w[:], t3[:], -0.5, 1.5, OP.mult, OP.add)
                al2 = spool.tile([P, g_tiles], F32, tag=f"nw{it}d")
                nc.vector.tensor_tensor(al2[:], al[:], w[:], OP.mult)
                al = al2
            alpha = al
            beta = spool.tile([P, g_tiles], F32, tag="beta")
            nc.vector.scalar_tensor_tensor(
                beta[:], mean[:], -1.0, alpha[:], OP.mult, OP.mult
            )
            # tanh pass coefficients: a_t = m_t + 0.5 m_s
            a_t = spool.tile([P, g_tiles], F32, tag="a_t")
            nc.vector.scalar_tensor_tensor(a_t[:], m_s, 0.5, m_t, OP.mult, OP.add)
            s_t = spool.tile([P, g_tiles], F32, tag="s_t")
            nc.vector.tensor_tensor(s_t[:], a_t[:], alpha[:], OP.mult)
            b_t = spool.tile([P, g_tiles], F32, tag="b_t")
            nc.vector.tensor_tensor(b_t[:], a_t[:], beta[:], OP.mult)
            # gelu pass coefficients: u2 = m_g + KF m_r
            u2 = spool.tile([P, g_tiles], F32, tag="u2")
            nc.vector.scalar_tensor_tensor(u2[:], m_r, KF, m_g, OP.mult, OP.add)
            s_g = spool.tile([P, g_tiles], F32, tag="s_g")
            nc.vector.tensor_tensor(s_g[:], u2[:], alpha[:], OP.mult)
            t4 = spool.tile([P, g_tiles], F32, tag="t4")
            nc.vector.tensor_tensor(t4[:], u2[:], beta[:], OP.mult)
            b_g = spool.tile([P, g_tiles], F32, tag="b_g")
            nc.vector.scalar_tensor_tensor(b_g[:], m_s, B_STAR, t4[:], OP.mult, OP.add)
            # output weight for the gelu pass: W = m_g + m_s + m_r/KF
            wv3 = spool.tile([P, g_tiles], F32, tag="wv3")
            nc.vector.scalar_tensor_tensor(wv3[:], m_r, 1.0 / KF, m_s, OP.mult, OP.add)
            wfin = spool.tile([P, g_tiles], F32, tag="wfin")
            nc.vector.tensor_tensor(wfin[:], wv3[:], m_g, OP.add)

            og_t = opool.tile([P, g_tiles, D], F32, tag="ot")
            for g in range(g_tiles):
                xs = xt[:, g, :]
                pg = apool.tile([P, D], F32, tag="pg")
                nc.scalar.activation(
                    pg[:], xs, AF.Gelu_apprx_tanh,
                    bias=b_g[:, g : g + 1], scale=s_g[:, g : g + 1],
                )
                th = apool.tile([P, D], F32, tag="th")
                nc.scalar.activation(
                    th[:], xs, AF.Tanh,
                    bias=b_t[:, g : g + 1], scale=s_t[:, g : g + 1],
                )
                c2 = apool.tile([P, D], F32, tag="c2")
                nc.gpsimd.tensor_scalar(
                    c2[:], th[:], a_t[:, g : g + 1], None, OP.mult
                )
                nc.vector.scalar_tensor_tensor(
                    og_t[:, g, :], pg[:], wfin[:, g : g + 1], c2[:],
                    OP.mult, OP.add,
                )
            nc.sync.dma_start(og[n].rearrange("g p d -> p g d"), og_t[:])

    nc.compile()
    return nc


_nc_cache = {}


def _get_nc(rows):
    if rows not in _nc_cache:
        _nc_cache[rows] = build_program(rows)
    return _nc_cache[rows]


def make_const_inputs(centroids):
    c = np.asarray(centroids, dtype=np.float32)
    ct = np.zeros((P, KCHUNKS * (C + 1)), np.float32)
    for k in range(KCHUNKS):
        ct[:, k * (C + 1) : k * (C + 1) + C] = c[:, k * P : (k + 1) * P].T
        ct[:, k * (C + 1) + C] = 1.0
    negn = np.zeros((P, C + 1), np.float32)
    negn[:, :C] = (-0.5 * np.sum(c.astype(np.float64) ** 2, axis=1)).astype(np.float32)
    ident = np.eye(P, dtype=np.float32)
    return ct, negn, ident


def run_sharded(x, centroids, **spmd_kwargs):
    x = np.ascontiguousarray(np.asarray(x), dtype=np.float32)
    assert x.shape == (N_TOTAL, D)
    nc = _get_nc(ROWS)
    ct, negn, ident = make_const_inputs(centroids)
    in_maps = []
    for ci in range(N_CORES):
        shard = np.zeros((ROWS, D), np.float32)
        shard[:ROWS_REAL] = x[ci * ROWS_REAL : (ci + 1) * ROWS_REAL]
        in_maps.append({"x": shard, "ct": ct, "negn": negn, "ident": ident})
    res = bass_utils.run_bass_kernel_spmd(
        nc, in_maps, core_ids=list(range(N_CORES)), **spmd_kwargs
    )
    out = np.concatenate([r["out"][:ROWS_REAL] for r in res.results], axis=0)
    return out, res


def kernel(x, centroids):
    out, _ = run_sharded(x, centroids)
    return out


# revision 5
# speedup vs baseline: 1.9002x; 1.9002x over previous
"""Trainium2 Bass kernel: ClusterActivation (nearest-centroid routing +
per-row normalization + per-cluster activation).

Data-parallel over 8 NeuronCores: x sharded along rows (padded to a
multiple of 128*G), centroids replicated. Per core, rows are processed
in [128, 512] tiles, G tiles per group:

  - nearest centroid: PE transposes the x tile (f32 exact), ScalarE
    copies PSUM->SBUF, then f32 matmuls against centroid chunks
    accumulate [x @ c^T | row_sum] in PSUM (a 5th ones-column gives the
    row sum for free). A DVE scalar_tensor_tensor adds -|c_j|^2/2;
    argmax via reduce + is_equal gives one-hot masks.
  - stats: sum(x^2) via a DVE scalar_tensor_tensor with accum_out;
    mean from the PE ones-column; inv_std = rsqrt via Newton on DVE.
  - activations: two ScalarE passes from ONE LUT set
    (gelu_apprx_tanh_and_others):
      pass G: Gelu_apprx_tanh(x*sg + bg) covers gelu rows (scale α),
        relu rows (scale KF*α; gelu(KF z)/KF == relu(z) for large KF —
        LUT tail verified exact on HW), and sigmoid rows (scale 0,
        bias b* with gelu(b*)=0.5).
      pass T: Tanh(x*st + bt) covers tanh rows (a=1) and sigmoid rows
        (a=0.5; sigmoid(z) = 0.5 tanh(z/2) + 0.5).
    GpSimd computes c2 = tanh_out * a_t; DVE combines
    out = pg * W + c2 in one scalar_tensor_tensor.
  - per-row coefficient algebra is batched across the G tiles of a
    group ([128, G] ops), amortizing DVE instruction overhead.
"""

import numpy as np
from contextlib import ExitStack

import concourse.bacc as bacc
import concourse.mybir as mybir
from concourse.tile import TileContext
from concourse import bass_utils

F32 = mybir.dt.float32
AF = mybir.ActivationFunctionType
OP = mybir.AluOpType
AX = mybir.AxisListType

N_TOTAL, D, C = 100000, 512, 4
N_CORES = 8
P = 128
G = 7  # tiles per group
ROWS = 12544  # padded rows per core: 98 tiles = 14 groups of 7
ROWS_REAL = N_TOTAL // N_CORES  # 12500
KCHUNKS = D // P  # 4
EPS = 1e-5
KF = 1024.0  # relu-via-gelu factor
INV_N = 1.0 / D
INV_NM1 = 1.0 / (D - 1.0)
BIAS_N = D * INV_NM1  # 512/511


def _solve_bstar():
    """b with gelu_tanh(b) == 0.5."""
    def g(z):
        return 0.5 * z * (1 + np.tanh(np.sqrt(2 / np.pi) * (z + 0.044715 * z**3)))

    lo, hi = 0.1, 1.5
    for _ in range(200):
        mid = 0.5 * (lo + hi)
        if g(mid) < 0.5:
            lo = mid
        else:
            hi = mid
    return float(0.5 * (lo + hi))


B_STAR = _solve_bstar()


def build_program(rows=ROWS, g_tiles=G):
    assert rows % (P * g_tiles) == 0
    n_groups = rows // (P * g_tiles)
    nc = bacc.Bacc("TRN2", target_bir_lowering=False, debug=False)
    x = nc.dram_tensor("x", [rows, D], F32, kind="ExternalInput").ap()
    # ct[d, k*(C+1) + j] = centroids[j, k*128+d] for j<C; ones for j=C
    ct = nc.dram_tensor("ct", [P, KCHUNKS * (C + 1)], F32, kind="ExternalInput").ap()
    # negn_rep[p, j] = -|c_j|^2/2 for j<C, 0 for j=C (replicated rows)
    negn = nc.dram_tensor("negn", [P, C + 1], F32, kind="ExternalInput").ap()
    ident = nc.dram_tensor("ident", [P, P], F32, kind="ExternalInput").ap()
    out = nc.dram_tensor("out", [rows, D], F32, kind="ExternalOutput").ap()

    with ExitStack() as ctx:
        tc = ctx.enter_context(TileContext(nc))
        cpool = ctx.enter_context(tc.tile_pool(name="const", bufs=1))
        xpool = ctx.enter_context(tc.tile_pool(name="xin", bufs=2))
        xtpool = ctx.enter_context(tc.tile_pool(name="xt", bufs=3))
        scpool = ctx.enter_context(tc.tile_pool(name="scratch", bufs=2))
        apool = ctx.enter_context(tc.tile_pool(name="acts", bufs=3))
        opool = ctx.enter_context(tc.tile_pool(name="outs", bufs=2))
        spool = ctx.enter_context(tc.tile_pool(name="small", bufs=2))
        ptpool = ctx.enter_context(tc.tile_pool(name="pxt", bufs=3, space="PSUM"))
        pspool = ctx.enter_context(tc.tile_pool(name="psc", bufs=3, space="PSUM"))

        ct_sb = cpool.tile([P, KCHUNKS, C + 1], F32)
        nc.sync.dma_start(ct_sb[:], ct.rearrange("d (k j) -> d k j", k=KCHUNKS))
        id_sb = cpool.tile([P, P], F32)
        nc.sync.dma_start(id_sb[:], ident)
        negn_sb = cpool.tile([P, C + 1], F32)
        nc.sync.dma_start(negn_sb[:], negn)

        xg = x.rearrange("(n g p) d -> n g p d", g=g_tiles, p=P)
        og = out.rearrange("(n g p) d -> n g p d", g=g_tiles, p=P)

        for n in range(n_groups):
            xt = xpool.tile([P, g_tiles, D], F32, tag="x")
            nc.sync.dma_start(xt[:], xg[n].rearrange("g p d -> p g d"))

            # group arrays
            adj = spool.tile([P, g_tiles, C + 1], F32, tag="adj")
            ssq = spool.tile([P, g_tiles], F32, tag="ssq")
            smax = spool.tile([P, g_tiles], F32, tag="smax")
            masks = spool.tile([P, g_tiles, C], F32, tag="masks")

            for g in range(g_tiles):
                xs = xt[:, g, :]
                # transpose x tile -> PSUM -> SBUF
                pxt = ptpool.tile([P, KCHUNKS, P], F32, tag="pxt")
                for k in range(KCHUNKS):
                    nc.tensor.transpose(
                        pxt[:, k, :], xs[:, k * P : (k + 1) * P], id_sb[:]
                    )
                xtsb = xtpool.tile([P, KCHUNKS, P], F32, tag="xtsb")
                nc.scalar.copy(xtsb[:], pxt[:])
                # scores + row-sum
                psc = pspool.tile([P, C + 1], F32, tag="psc")
                for k in range(KCHUNKS):
                    nc.tensor.matmul(
                        psc[:],
                        lhsT=xtsb[:, k, :],
                        rhs=ct_sb[:, k, :],
                        start=(k == 0),
                        stop=(k == KCHUNKS - 1),
                    )
                # adj = psc + [-|c|^2/2, 0]
                nc.vector.scalar_tensor_tensor(
                    adj[:, g, :], psc[:], 1.0, negn_sb[:], OP.mult, OP.add
                )
                # sum(x^2) with accumulate
                sq = scpool.tile([P, D], F32, tag="sq")
                nc.vector.scalar_tensor_tensor(
                    sq[:], xs, 1.0, xs, OP.bypass, OP.mult,
                    accum_out=ssq[:, g : g + 1],
                )
                nc.vector.tensor_reduce(
                    smax[:, g : g + 1], adj[:, g, 0:C], axis=AX.X, op=OP.max
                )
                nc.vector.tensor_scalar(
                    masks[:, g, :], adj[:, g, 0:C], smax[:, g : g + 1], None,
                    OP.is_equal,
                )

            # ---- batched per-row coefficient algebra ([P, G] ops) ----
            m_r = masks[:, :, 0]
            m_t = masks[:, :, 1]
            m_g = masks[:, :, 2]
            m_s = masks[:, :, 3]
            mean = spool.tile([P, g_tiles], F32, tag="mean")
            nc.vector.tensor_scalar(mean[:], adj[:, :, C], INV_N, None, OP.mult)
            msq = spool.tile([P, g_tiles], F32, tag="msq")
            nc.vector.tensor_tensor(msq[:], mean[:], mean[:], OP.mult)
            u = spool.tile([P, g_tiles], F32, tag="u")
            nc.vector.tensor_scalar(u[:], msq[:], BIAS_N, -EPS, OP.mult, OP.add)
            vv = spool.tile([P, g_tiles], F32, tag="vv")
            nc.vector.scalar_tensor_tensor(
                vv[:], ssq[:], INV_NM1, u[:], OP.mult, OP.subtract
            )
            al = spool.tile([P, g_tiles], F32, tag="seed")
            nc.vector.tensor_scalar(al[:], vv[:], -0.5, 1.5, OP.mult, OP.add)
            for it in range(2):
                t2 = spool.tile([P, g_tiles], F32, tag=f"nw{it}a")
                nc.vector.tensor_tensor(t2[:], al[:], al[:], OP.mult)
                t3 = spool.tile([P, g_tiles], F32, tag=f"nw{it}b")
                nc.vector.tensor_tensor(t3[:], t2[:], vv[:], OP.mult)
                w = spool.tile([P, g_tiles], F32, tag=f"nw{it}c")
                nc.vector.tensor_scalar(w[:], t3[:], -0.5, 1.5, OP.mult, OP.add)
                al2 = spool.tile([P, g_tiles], F32, tag=f"nw{it}d")
                nc.vector.tensor_tensor(al2[:], al[:], w[:], OP.mult)
                al = al2
            alpha = al
            beta = spool.tile([P, g_tiles], F32, tag="beta")
            nc.vector.scalar_tensor_tensor(
                beta[:], mean[:], -1.0, alpha[:], OP.mult, OP.mult
            )
            # tanh pass coefficients: a_t = m_t + 0.5 m_s
            a_t = spool.tile([P, g_tiles], F32, tag="a_t")
            nc.vector.scalar_tensor_tensor(a_t[:], m_s, 0.5, m_t, OP.mult, OP.add)
            s_t = spool.tile([P, g_tiles], F32, tag="s_t")
            nc.vector.tensor_tensor(s_t[:], a_t[:], alpha[:], OP.mult)
            b_t = spool.tile([P, g_tiles], F32, tag="b_t")
            nc.vector.tensor_tensor(b_t[:], a_t[:], beta[:], OP.mult)
            # gelu pass coefficients: u2 = m_g + KF m_r
            u2 = spool.tile([P, g_tiles], F32, tag="u2")
            nc.vector.scalar_tensor_tensor(u2[:], m_r, KF, m_g, OP.mult, OP.add)
            s_g = spool.tile([P, g_tiles], F32, tag="s_g")
            nc.vector.tensor_tensor(s_g[:], u2[:], alpha[:], OP.mult)
            t4 = spool.tile([P, g_tiles], F32, tag="t4")
            nc.vector.tensor_tensor(t4[:], u2[:], beta[:], OP.mult)
            b_g = spool.tile([P, g_tiles], F32, tag="b_g")
            nc.vector.scalar_tensor_tensor(b_g[:], m_s, B_STAR, t4[:], OP.mult, OP.add)
            # output weight for the gelu pass: W = m_g + m_s + m_r/KF
            wv3 = spool.tile([P, g_tiles], F32, tag="wv3")
            nc.vector.scalar_tensor_tensor(wv3[:], m_r, 1.0 / KF, m_s, OP.mult, OP.add)
            wfin = spool.tile([P, g_tiles], F32, tag="wfin")
            nc.vector.tensor_tensor(wfin[:], wv3[:], m_g, OP.add)

            og_t = opool.tile([P, g_tiles, D], F32, tag="ot")
            for g in range(g_tiles):
                xs = xt[:, g, :]
                pg = apool.tile([P, D], F32, tag="pg")
                nc.scalar.activation(
                    pg[:], xs, AF.Gelu_apprx_tanh,
                    bias=b_g[:, g : g + 1], scale=s_g[:, g : g + 1],
                )
                th = apool.tile([P, D], F32, tag="th")
                nc.scalar.activation(
                    th[:], xs, AF.Tanh,
                    bias=b_t[:, g : g + 1], scale=s_t[:, g : g + 1],
                )
                c2 = apool.tile([P, D], F32, tag="c2")
                nc.vector.tensor_scalar(
                    c2[:], th[:], a_t[:, g : g + 1], None, OP.mult
                )
                nc.vector.scalar_tensor_tensor(
                    og_t[:, g, :], pg[:], wfin[:, g : g + 1], c2[:],
                    OP.mult, OP.add,
                )
            nc.sync.dma_start(og[n].rearrange("g p d -> p g d"), og_t[:])

    nc.compile()
    return nc


_nc_cache = {}


def _get_nc(rows):
    if rows not in _nc_cache:
        _nc_cache[rows] = build_program(rows)
    return _nc_cache[rows]


def make_const_inputs(centroids):
    c = np.asarray(centroids, dtype=np.float32)
    ct = np.zeros((P, KCHUNKS * (C + 1)), np.float32)
    for k in range(KCHUNKS):
        ct[:, k * (C + 1) : k * (C + 1) + C] = c[:, k * P : (k + 1) * P].T
        ct[:, k * (C + 1) + C] = 1.0
    negn = np.zeros((P, C + 1), np.float32)
    negn[:, :C] = (-0.5 * np.sum(c.astype(np.float64) ** 2, axis=1)).astype(np.float32)
    ident = np.eye(P, dtype=np.float32)
    return ct, negn, ident


def run_sharded(x, centroids, **spmd_kwargs):
    x = np.ascontiguousarray(np.asarray(x), dtype=np.float32)
    assert x.shape == (N_TOTAL, D)
    nc = _get_nc(ROWS)
    ct, negn, ident = make_const_inputs(centroids)
    in_maps = []
    for ci in range(N_CORES):
        shard = np.zeros((ROWS, D), np.float32)
        shard[:ROWS_REAL] = x[ci * ROWS_REAL : (ci + 1) * ROWS_REAL]
        in_maps.append({"x": shard, "ct": ct, "negn": negn, "ident": ident})
    res = bass_utils.run_bass_kernel_spmd(
        nc, in_maps, core_ids=list(range(N_CORES)), **spmd_kwargs
    )
    out = np.concatenate([r["out"][:ROWS_REAL] for r in res.results], axis=0)
    return out, res


def kernel(x, centroids):
    out, _ = run_sharded(x, centroids)
    return out


# revision 6
# speedup vs baseline: 2.2088x; 1.1624x over previous
"""Trainium2 Bass kernel: ClusterActivation (nearest-centroid routing +
per-row normalization + per-cluster activation).

Data-parallel over 8 NeuronCores: x sharded along rows (padded to a
multiple of 128*G), centroids replicated. Per core, rows are processed
in [128, 512] tiles, G tiles per group:

  - nearest centroid: PE transposes the x tile (f32 exact), ScalarE
    copies PSUM->SBUF, then f32 matmuls against centroid chunks
    accumulate [x @ c^T | row_sum] in PSUM (a 5th ones-column gives the
    row sum for free). A DVE scalar_tensor_tensor adds -|c_j|^2/2;
    argmax via reduce + is_equal gives one-hot masks.
  - stats: sum(x^2) via a DVE scalar_tensor_tensor with accum_out;
    mean from the PE ones-column; inv_std = rsqrt via Newton on DVE.
  - activations: two ScalarE passes from ONE LUT set
    (gelu_apprx_tanh_and_others):
      pass G: Gelu_apprx_tanh(x*sg + bg) covers gelu rows (scale α),
        relu rows (scale KF*α; gelu(KF z)/KF == relu(z) for large KF —
        LUT tail verified exact on HW), and sigmoid rows (scale 0,
        bias b* with gelu(b*)=0.5).
      pass T: Tanh(x*st + bt) covers tanh rows (a=1) and sigmoid rows
        (a=0.5; sigmoid(z) = 0.5 tanh(z/2) + 0.5).
    GpSimd computes c2 = tanh_out * a_t; DVE combines
    out = pg * W + c2 in one scalar_tensor_tensor.
  - per-row coefficient algebra is batched across the G tiles of a
    group ([128, G] ops), amortizing DVE instruction overhead.
"""

import numpy as np
from contextlib import ExitStack

import concourse.bacc as bacc
import concourse.mybir as mybir
from concourse.tile import TileContext
from concourse import bass_utils

F32 = mybir.dt.float32
AF = mybir.ActivationFunctionType
OP = mybir.AluOpType
AX = mybir.AxisListType

N_TOTAL, D, C = 100000, 512, 4
N_CORES = 8
P = 128
G = 14  # tiles per algebra/DMA window
ROWS = 12544  # padded rows per core: 98 tiles = 7 windows of 14
ROWS_REAL = N_TOTAL // N_CORES  # 12500
KCHUNKS = D // P  # 4
EPS = 1e-5
KF = 1024.0  # relu-via-gelu factor
INV_N = 1.0 / D
INV_NM1 = 1.0 / (D - 1.0)
BIAS_N = D * INV_NM1  # 512/511


def _solve_bstar():
    """b with gelu_tanh(b) == 0.5."""
    def g(z):
        return 0.5 * z * (1 + np.tanh(np.sqrt(2 / np.pi) * (z + 0.044715 * z**3)))

    lo, hi = 0.1, 1.5
    for _ in range(200):
        mid = 0.5 * (lo + hi)
        if g(mid) < 0.5:
            lo = mid
        else:
            hi = mid
    return float(0.5 * (lo + hi))


B_STAR = _solve_bstar()


def build_program(rows=ROWS, g_tiles=G):
    assert rows % (P * g_tiles) == 0
    n_groups = rows // (P * g_tiles)
    nc = bacc.Bacc("TRN2", target_bir_lowering=False, debug=False)
    x = nc.dram_tensor("x", [rows, D], F32, kind="ExternalInput").ap()
    # ct[d, k*(C+1) + j] = centroids[j, k*128+d] for j<C; ones for j=C
    ct = nc.dram_tensor("ct", [P, KCHUNKS * (C + 1)], F32, kind="ExternalInput").ap()
    # negn_rep[p, j] = -|c_j|^2/2 for j<C, 0 for j=C (replicated rows)
    negn = nc.dram_tensor("negn", [P, C + 1], F32, kind="ExternalInput").ap()
    ident = nc.dram_tensor("ident", [P, P], F32, kind="ExternalInput").ap()
    out = nc.dram_tensor("out", [rows, D], F32, kind="ExternalOutput").ap()

    with ExitStack() as ctx:
        tc = ctx.enter_context(TileContext(nc))
        cpool = ctx.enter_context(tc.tile_pool(name="const", bufs=1))
        xpool = ctx.enter_context(tc.tile_pool(name="xin", bufs=2))
        xtpool = ctx.enter_context(tc.tile_pool(name="xt", bufs=4))
        scpool = ctx.enter_context(tc.tile_pool(name="scratch", bufs=2))
        apool = ctx.enter_context(tc.tile_pool(name="acts", bufs=4))
        opool = ctx.enter_context(tc.tile_pool(name="outs", bufs=2))
        spool = ctx.enter_context(tc.tile_pool(name="small", bufs=2))
        ptpool = ctx.enter_context(tc.tile_pool(name="pxt", bufs=3, space="PSUM"))
        pspool = ctx.enter_context(tc.tile_pool(name="psc", bufs=4, space="PSUM"))

        ct_sb = cpool.tile([P, KCHUNKS, C + 1], F32)
        nc.sync.dma_start(ct_sb[:], ct.rearrange("d (k j) -> d k j", k=KCHUNKS))
        id_sb = cpool.tile([P, P], F32)
        nc.sync.dma_start(id_sb[:], ident)
        negn_sb = cpool.tile([P, C + 1], F32)
        nc.sync.dma_start(negn_sb[:], negn)

        xg = x.rearrange("(n g p) d -> n g p d", g=g_tiles, p=P)
        og = out.rearrange("(n g p) d -> n g p d", g=g_tiles, p=P)

        for n in range(n_groups):
            xt = xpool.tile([P, g_tiles, D], F32, tag="x")
            nc.sync.dma_start(xt[:], xg[n].rearrange("g p d -> p g d"))

            # group arrays
            adj = spool.tile([P, g_tiles, C + 1], F32, tag="adj")
            ssq = spool.tile([P, g_tiles], F32, tag="ssq")
            smax = spool.tile([P, g_tiles], F32, tag="smax")
            masks = spool.tile([P, g_tiles, C], F32, tag="masks")

            for g in range(g_tiles):
                xs = xt[:, g, :]
                # transpose x tile -> PSUM -> SBUF
                pxt = ptpool.tile([P, KCHUNKS, P], F32, tag="pxt")
                for k in range(KCHUNKS):
                    nc.tensor.transpose(
                        pxt[:, k, :], xs[:, k * P : (k + 1) * P], id_sb[:]
                    )
                xtsb = xtpool.tile([P, KCHUNKS, P], F32, tag="xtsb")
                nc.scalar.copy(xtsb[:], pxt[:])
                # scores + row-sum
                psc = pspool.tile([P, C + 1], F32, tag="psc")
                for k in range(KCHUNKS):
                    nc.tensor.matmul(
                        psc[:],
                        lhsT=xtsb[:, k, :],
                        rhs=ct_sb[:, k, :],
                        start=(k == 0),
                        stop=(k == KCHUNKS - 1),
                    )
                # adj = psc + [-|c|^2/2, 0]
                nc.vector.scalar_tensor_tensor(
                    adj[:, g, :], psc[:], 1.0, negn_sb[:], OP.mult, OP.add
                )
                # sum(x^2) with accumulate
                sq = scpool.tile([P, D], F32, tag="sq")
                nc.vector.scalar_tensor_tensor(
                    sq[:], xs, 1.0, xs, OP.bypass, OP.mult,
                    accum_out=ssq[:, g : g + 1],
                )
                nc.vector.tensor_reduce(
                    smax[:, g : g + 1], adj[:, g, 0:C], axis=AX.X, op=OP.max
                )
                nc.vector.tensor_scalar(
                    masks[:, g, :], adj[:, g, 0:C], smax[:, g : g + 1], None,
                    OP.is_equal,
                )

            # ---- batched per-row coefficient algebra ([P, G] ops) ----
            m_r = masks[:, :, 0]
            m_t = masks[:, :, 1]
            m_g = masks[:, :, 2]
            m_s = masks[:, :, 3]
            mean = spool.tile([P, g_tiles], F32, tag="mean")
            nc.vector.tensor_scalar(mean[:], adj[:, :, C], INV_N, None, OP.mult)
            msq = spool.tile([P, g_tiles], F32, tag="msq")
            nc.vector.tensor_tensor(msq[:], mean[:], mean[:], OP.mult)
            u = spool.tile([P, g_tiles], F32, tag="u")
            nc.vector.tensor_scalar(u[:], msq[:], BIAS_N, -EPS, OP.mult, OP.add)
            vv = spool.tile([P, g_tiles], F32, tag="vv")
            nc.vector.scalar_tensor_tensor(
                vv[:], ssq[:], INV_NM1, u[:], OP.mult, OP.subtract
            )
            al = spool.tile([P, g_tiles], F32, tag="seed")
            nc.vector.tensor_scalar(al[:], vv[:], -0.5, 1.5, OP.mult, OP.add)
            for it in range(2):
                t2 = spool.tile([P, g_tiles], F32, tag=f"nw{it}a")
                nc.vector.tensor_tensor(t2[:], al[:], al[:], OP.mult)
                t3 = spool.tile([P, g_tiles], F32, tag=f"nw{it}b")
                nc.vector.tensor_tensor(t3[:], t2[:], vv[:], OP.mult)
                w = spool.tile([P, g_tiles], F32, tag=f"nw{it}c")
                nc.vector.tensor_scalar(w[:], t3[:], -0.5, 1.5, OP.mult, OP.add)
                al2 = spool.tile([P, g_tiles], F32, tag=f"nw{it}d")
                nc.vector.tensor_tensor(al2[:], al[:], w[:], OP.mult)
                al = al2
            alpha = al
            beta = spool.tile([P, g_tiles], F32, tag="beta")
            nc.vector.scalar_tensor_tensor(
                beta[:], mean[:], -1.0, alpha[:], OP.mult, OP.mult
            )
            # tanh pass coefficients: a_t = m_t + 0.5 m_s
            a_t = spool.tile([P, g_tiles], F32, tag="a_t")
            nc.vector.scalar_tensor_tensor(a_t[:], m_s, 0.5, m_t, OP.mult, OP.add)
            s_t = spool.tile([P, g_tiles], F32, tag="s_t")
            nc.vector.tensor_tensor(s_t[:], a_t[:], alpha[:], OP.mult)
            b_t = spool.tile([P, g_tiles], F32, tag="b_t")
            nc.vector.tensor_tensor(b_t[:], a_t[:], beta[:], OP.mult)
            # gelu pass coefficients: u2 = m_g + KF m_r
            u2 = spool.tile([P, g_tiles], F32, tag="u2")
            nc.vector.scalar_tensor_tensor(u2[:], m_r, KF, m_g, OP.mult, OP.add)
            s_g = spool.tile([P, g_tiles], F32, tag="s_g")
            nc.vector.tensor_tensor(s_g[:], u2[:], alpha[:], OP.mult)
            t4 = spool.tile([P, g_tiles], F32, tag="t4")
            nc.vector.tensor_tensor(t4[:], u2[:], beta[:], OP.mult)
            b_g = spool.tile([P, g_tiles], F32, tag="b_g")
            nc.vector.scalar_tensor_tensor(b_g[:], m_s, B_STAR, t4[:], OP.mult, OP.add)
            # output weight for the gelu pass: W = m_g + m_s + m_r/KF
            wv3 = spool.tile([P, g_tiles], F32, tag="wv3")
            nc.vector.scalar_tensor_tensor(wv3[:], m_r, 1.0 / KF, m_s, OP.mult, OP.add)
            wfin = spool.tile([P, g_tiles], F32, tag="wfin")
            nc.vector.tensor_tensor(wfin[:], wv3[:], m_g, OP.add)

            og_t = opool.tile([P, g_tiles, D], F32, tag="ot")
            for g in range(g_tiles):
                xs = xt[:, g, :]
                pg = apool.tile([P, D], F32, tag="pg")
                nc.scalar.activation(
                    pg[:], xs, AF.Gelu_apprx_tanh,
                    bias=b_g[:, g : g + 1], scale=s_g[:, g : g + 1],
                )
                th = apool.tile([P, D], F32, tag="th")
                nc.scalar.activation(
                    th[:], xs, AF.Tanh,
                    bias=b_t[:, g : g + 1], scale=s_t[:, g : g + 1],
                )
                c2 = apool.tile([P, D], F32, tag="c2")
                nc.vector.tensor_scalar(
                    c2[:], th[:], a_t[:, g : g + 1], None, OP.mult
                )
                nc.vector.scalar_tensor_tensor(
                    og_t[:, g, :], pg[:], wfin[:, g : g + 1], c2[:],
                    OP.mult, OP.add,
                )
            nc.sync.dma_start(og[n].rearrange("g p d -> p g d"), og_t[:])

    nc.compile()
    return nc


_nc_cache = {}


def _get_nc(rows):
    if rows not in _nc_cache:
        _nc_cache[rows] = build_program(rows)
    return _nc_cache[rows]


def make_const_inputs(centroids):
    c = np.asarray(centroids, dtype=np.float32)
    ct = np.zeros((P, KCHUNKS * (C + 1)), np.float32)
    for k in range(KCHUNKS):
        ct[:, k * (C + 1) : k * (C + 1) + C] = c[:, k * P : (k + 1) * P].T
        ct[:, k * (C + 1) + C] = 1.0
    negn = np.zeros((P, C + 1), np.float32)
    negn[:, :C] = (-0.5 * np.sum(c.astype(np.float64) ** 2, axis=1)).astype(np.float32)
    ident = np.eye(P, dtype=np.float32)
    return ct, negn, ident


def run_sharded(x, centroids, **spmd_kwargs):
    x = np.ascontiguousarray(np.asarray(x), dtype=np.float32)
    assert x.shape == (N_TOTAL, D)
    nc = _get_nc(ROWS)
    ct, negn, ident = make_const_inputs(centroids)
    in_maps = []
    for ci in range(N_CORES):
        shard = np.zeros((ROWS, D), np.float32)
        shard[:ROWS_REAL] = x[ci * ROWS_REAL : (ci + 1) * ROWS_REAL]
        in_maps.append({"x": shard, "ct": ct, "negn": negn, "ident": ident})
    res = bass_utils.run_bass_kernel_spmd(
        nc, in_maps, core_ids=list(range(N_CORES)), **spmd_kwargs
    )
    out = np.concatenate([r["out"][:ROWS_REAL] for r in res.results], axis=0)
    return out, res


def kernel(x, centroids):
    out, _ = run_sharded(x, centroids)
    return out


# revision 8
# speedup vs baseline: 2.3623x; 1.0695x over previous
"""Trainium2 Bass kernel: ClusterActivation (nearest-centroid routing +
per-row normalization + per-cluster activation).

Data-parallel over 8 NeuronCores: x sharded along rows (padded to a
multiple of 128*G), centroids replicated. Per core, rows are processed
in [128, 512] tiles, G tiles per group:

  - nearest centroid: PE transposes the x tile (f32 exact), ScalarE
    copies PSUM->SBUF, then f32 matmuls against centroid chunks
    accumulate [x @ c^T | row_sum] in PSUM (a 5th ones-column gives the
    row sum for free). A DVE scalar_tensor_tensor adds -|c_j|^2/2;
    argmax via reduce + is_equal gives one-hot masks.
  - stats: sum(x^2) via a DVE scalar_tensor_tensor with accum_out;
    mean from the PE ones-column; inv_std = rsqrt via Newton on DVE.
  - activations: two ScalarE passes from ONE LUT set
    (gelu_apprx_tanh_and_others):
      pass G: Gelu_apprx_tanh(x*sg + bg) covers gelu rows (scale α),
        relu rows (scale KF*α; gelu(KF z)/KF == relu(z) for large KF —
        LUT tail verified exact on HW), and sigmoid rows (scale 0,
        bias b* with gelu(b*)=0.5).
      pass T: Tanh(x*st + bt) covers tanh rows (a=1) and sigmoid rows
        (a=0.5; sigmoid(z) = 0.5 tanh(z/2) + 0.5).
    GpSimd computes c2 = tanh_out * a_t; DVE combines
    out = pg * W + c2 in one scalar_tensor_tensor.
  - per-row coefficient algebra is batched across the G tiles of a
    group ([128, G] ops), amortizing DVE instruction overhead.
"""

import numpy as np
from contextlib import ExitStack

import concourse.bacc as bacc
import concourse.mybir as mybir
from concourse.tile import TileContext
from concourse import bass_utils

F32 = mybir.dt.float32
AF = mybir.ActivationFunctionType
OP = mybir.AluOpType
AX = mybir.AxisListType

N_TOTAL, D, C = 100000, 512, 4
N_CORES = 8
P = 128
G = 14  # tiles per algebra/DMA window
ROWS = 12544  # padded rows per core: 98 tiles = 7 windows of 14
ROWS_REAL = N_TOTAL // N_CORES  # 12500
KCHUNKS = D // P  # 4
EPS = 1e-5
KF = 1024.0  # relu-via-gelu factor
INV_N = 1.0 / D
INV_NM1 = 1.0 / (D - 1.0)
BIAS_N = D * INV_NM1  # 512/511


def _solve_bstar():
    """b with gelu_tanh(b) == 0.5."""
    def g(z):
        return 0.5 * z * (1 + np.tanh(np.sqrt(2 / np.pi) * (z + 0.044715 * z**3)))

    lo, hi = 0.1, 1.5
    for _ in range(200):
        mid = 0.5 * (lo + hi)
        if g(mid) < 0.5:
            lo = mid
        else:
            hi = mid
    return float(0.5 * (lo + hi))


B_STAR = _solve_bstar()


def build_program(rows=ROWS, g_tiles=G):
    assert rows % (P * g_tiles) == 0
    n_groups = rows // (P * g_tiles)
    nc = bacc.Bacc("TRN2", target_bir_lowering=False, debug=False)
    x = nc.dram_tensor("x", [rows, D], F32, kind="ExternalInput").ap()
    # ct[d, k*(C+1) + j] = centroids[j, k*128+d] for j<C; ones for j=C
    ct = nc.dram_tensor("ct", [P, KCHUNKS * (C + 1)], F32, kind="ExternalInput").ap()
    # negn_rep[p, j] = -|c_j|^2/2 for j<C, 0 for j=C (replicated rows)
    negn = nc.dram_tensor("negn", [P, C + 1], F32, kind="ExternalInput").ap()
    ident = nc.dram_tensor("ident", [P, P], F32, kind="ExternalInput").ap()
    out = nc.dram_tensor("out", [rows, D], F32, kind="ExternalOutput").ap()

    with ExitStack() as ctx:
        tc = ctx.enter_context(TileContext(nc))
        cpool = ctx.enter_context(tc.tile_pool(name="const", bufs=1))
        xpool = ctx.enter_context(tc.tile_pool(name="xin", bufs=2))
        xtpool = ctx.enter_context(tc.tile_pool(name="xt", bufs=4))
        scpool = ctx.enter_context(tc.tile_pool(name="scratch", bufs=2))
        apool = ctx.enter_context(tc.tile_pool(name="acts", bufs=4))
        opool = ctx.enter_context(tc.tile_pool(name="outs", bufs=2))
        spool = ctx.enter_context(tc.tile_pool(name="small", bufs=2))
        ptpool = ctx.enter_context(tc.tile_pool(name="pxt", bufs=3, space="PSUM"))
        pspool = ctx.enter_context(tc.tile_pool(name="psc", bufs=4, space="PSUM"))

        ct_sb = cpool.tile([P, KCHUNKS, C + 1], F32)
        nc.sync.dma_start(ct_sb[:], ct.rearrange("d (k j) -> d k j", k=KCHUNKS))
        id_sb = cpool.tile([P, P], F32)
        nc.sync.dma_start(id_sb[:], ident)
        negn_sb = cpool.tile([P, C + 1], F32)
        nc.sync.dma_start(negn_sb[:], negn)

        xg = x.rearrange("(n g p) d -> n g p d", g=g_tiles, p=P)
        og = out.rearrange("(n g p) d -> n g p d", g=g_tiles, p=P)

        def part_a(n):
            """DMA + per-tile PE/stat work; returns state for algebra/part-B."""
            xt = xpool.tile([P, g_tiles, D], F32, tag="x")
            nc.sync.dma_start(xt[:], xg[n].rearrange("g p d -> p g d"))
            adj = spool.tile([P, g_tiles, C + 1], F32, tag="adj")
            ssq = spool.tile([P, g_tiles], F32, tag="ssq")
            smax = spool.tile([P, g_tiles], F32, tag="smax")
            masks = spool.tile([P, g_tiles, C], F32, tag="masks")
            return dict(xt=xt, adj=adj, ssq=ssq, smax=smax, masks=masks)

        def part_a_tile(st, g):
            xt, adj, ssq, smax, masks = (
                st["xt"], st["adj"], st["ssq"], st["smax"], st["masks"]
            )
            xs = xt[:, g, :]
            pxt = ptpool.tile([P, KCHUNKS, P], F32, tag="pxt")
            for k in range(KCHUNKS):
                nc.tensor.transpose(
                    pxt[:, k, :], xs[:, k * P : (k + 1) * P], id_sb[:]
                )
            xtsb = xtpool.tile([P, KCHUNKS, P], F32, tag="xtsb")
            nc.scalar.copy(xtsb[:], pxt[:])
            psc = pspool.tile([P, C + 1], F32, tag="psc")
            for k in range(KCHUNKS):
                nc.tensor.matmul(
                    psc[:],
                    lhsT=xtsb[:, k, :],
                    rhs=ct_sb[:, k, :],
                    start=(k == 0),
                    stop=(k == KCHUNKS - 1),
                )
            nc.vector.scalar_tensor_tensor(
                adj[:, g, :], psc[:], 1.0, negn_sb[:], OP.mult, OP.add
            )
            sq = scpool.tile([P, D], F32, tag="sq")
            nc.vector.scalar_tensor_tensor(
                sq[:], xs, 1.0, xs, OP.bypass, OP.mult,
                accum_out=ssq[:, g : g + 1],
            )
            nc.vector.tensor_reduce(
                smax[:, g : g + 1], adj[:, g, 0:C], axis=AX.X, op=OP.max
            )
            nc.vector.tensor_scalar(
                masks[:, g, :], adj[:, g, 0:C], smax[:, g : g + 1], None,
                OP.is_equal,
            )

        def algebra(st):
            adj, ssq, masks = st["adj"], st["ssq"], st["masks"]
            m_r = masks[:, :, 0]
            m_t = masks[:, :, 1]
            m_g = masks[:, :, 2]
            m_s = masks[:, :, 3]
            mean = spool.tile([P, g_tiles], F32, tag="mean")
            nc.vector.tensor_scalar(mean[:], adj[:, :, C], INV_N, None, OP.mult)
            msq = spool.tile([P, g_tiles], F32, tag="msq")
            nc.vector.tensor_tensor(msq[:], mean[:], mean[:], OP.mult)
            u = spool.tile([P, g_tiles], F32, tag="u")
            nc.vector.tensor_scalar(u[:], msq[:], BIAS_N, -EPS, OP.mult, OP.add)
            vv = spool.tile([P, g_tiles], F32, tag="vv")
            nc.vector.scalar_tensor_tensor(
                vv[:], ssq[:], INV_NM1, u[:], OP.mult, OP.subtract
            )
            al = spool.tile([P, g_tiles], F32, tag="seed")
            nc.vector.tensor_scalar(al[:], vv[:], -0.5, 1.5, OP.mult, OP.add)
            for it in range(2):
                t2 = spool.tile([P, g_tiles], F32, tag=f"nw{it}a")
                nc.vector.tensor_tensor(t2[:], al[:], al[:], OP.mult)
                t3 = spool.tile([P, g_tiles], F32, tag=f"nw{it}b")
                nc.vector.tensor_tensor(t3[:], t2[:], vv[:], OP.mult)
                w = spool.tile([P, g_tiles], F32, tag=f"nw{it}c")
                nc.vector.tensor_scalar(w[:], t3[:], -0.5, 1.5, OP.mult, OP.add)
                al2 = spool.tile([P, g_tiles], F32, tag=f"nw{it}d")
                nc.vector.tensor_tensor(al2[:], al[:], w[:], OP.mult)
                al = al2
            alpha = al
            beta = spool.tile([P, g_tiles], F32, tag="beta")
            nc.vector.scalar_tensor_tensor(
                beta[:], mean[:], -1.0, alpha[:], OP.mult, OP.mult
            )
            a_t = spool.tile([P, g_tiles], F32, tag="a_t")
            nc.vector.scalar_tensor_tensor(a_t[:], m_s, 0.5, m_t, OP.mult, OP.add)
            s_t = spool.tile([P, g_tiles], F32, tag="s_t")
            nc.vector.tensor_tensor(s_t[:], a_t[:], alpha[:], OP.mult)
            b_t = spool.tile([P, g_tiles], F32, tag="b_t")
            nc.vector.tensor_tensor(b_t[:], a_t[:], beta[:], OP.mult)
            u2 = spool.tile([P, g_tiles], F32, tag="u2")
            nc.vector.scalar_tensor_tensor(u2[:], m_r, KF, m_g, OP.mult, OP.add)
            s_g = spool.tile([P, g_tiles], F32, tag="s_g")
            nc.vector.tensor_tensor(s_g[:], u2[:], alpha[:], OP.mult)
            t4 = spool.tile([P, g_tiles], F32, tag="t4")
            nc.vector.tensor_tensor(t4[:], u2[:], beta[:], OP.mult)
            b_g = spool.tile([P, g_tiles], F32, tag="b_g")
            nc.vector.scalar_tensor_tensor(b_g[:], m_s, B_STAR, t4[:], OP.mult, OP.add)
            wv3 = spool.tile([P, g_tiles], F32, tag="wv3")
            nc.vector.scalar_tensor_tensor(wv3[:], m_r, 1.0 / KF, m_s, OP.mult, OP.add)
            wfin = spool.tile([P, g_tiles], F32, tag="wfin")
            nc.vector.tensor_tensor(wfin[:], wv3[:], m_g, OP.add)
            st["a_t"] = a_t
            st["s_t"] = s_t
            st["b_t"] = b_t
            st["s_g"] = s_g
            st["b_g"] = b_g
            st["wfin"] = wfin
            st["og_t"] = opool.tile([P, g_tiles, D], F32, tag="ot", name="og_t")

        def part_b_tile(st, g):
            xt = st["xt"]
            xs = xt[:, g, :]
            pg = apool.tile([P, D], F32, tag="pg")
            nc.scalar.activation(
                pg[:], xs, AF.Gelu_apprx_tanh,
                bias=st["b_g"][:, g : g + 1], scale=st["s_g"][:, g : g + 1],
            )
            th = apool.tile([P, D], F32, tag="th")
            nc.scalar.activation(
                th[:], xs, AF.Tanh,
                bias=st["b_t"][:, g : g + 1], scale=st["s_t"][:, g : g + 1],
            )
            c2 = apool.tile([P, D], F32, tag="c2")
            nc.vector.tensor_scalar(
                c2[:], th[:], st["a_t"][:, g : g + 1], None, OP.mult
            )
            nc.vector.scalar_tensor_tensor(
                st["og_t"][:, g, :], pg[:], st["wfin"][:, g : g + 1], c2[:],
                OP.mult, OP.add,
            )

        def part_b_flush(st, n):
            nc.sync.dma_start(og[n].rearrange("g p d -> p g d"), st["og_t"][:])

        # software-pipelined emission: window n part-A interleaved with
        # window n-1 part-B so every engine's queue alternates roles.
        prev = None
        for n in range(n_groups):
            st = part_a(n)
            for g in range(g_tiles):
                part_a_tile(st, g)
                if prev is not None:
                    part_b_tile(prev, g)
            if prev is not None:
                part_b_flush(prev, n - 1)
            algebra(st)
            prev = st
        for g in range(g_tiles):
            part_b_tile(prev, g)
        part_b_flush(prev, n_groups - 1)

    nc.compile()
    return nc


_nc_cache = {}


def _get_nc(rows):
    if rows not in _nc_cache:
        _nc_cache[rows] = build_program(rows)
    return _nc_cache[rows]


def make_const_inputs(centroids):
    c = np.asarray(centroids, dtype=np.float32)
    ct = np.zeros((P, KCHUNKS * (C + 1)), np.float32)
    for k in range(KCHUNKS):
        ct[:, k * (C + 1) : k * (C + 1) + C] = c[:, k * P : (k + 1) * P].T
        ct[:, k * (C + 1) + C] = 1.0
    negn = np.zeros((P, C + 1), np.float32)
    negn[:, :C] = (-0.5 * np.sum(c.astype(np.float64) ** 2, axis=1)).astype(np.float32)
    ident = np.eye(P, dtype=np.float32)
    return ct, negn, ident


def run_sharded(x, centroids, **spmd_kwargs):
    x = np.ascontiguousarray(np.asarray(x), dtype=np.float32)
    assert x.shape == (N_TOTAL, D)
    nc = _get_nc(ROWS)
    ct, negn, ident = make_const_inputs(centroids)
    in_maps = []
    for ci in range(N_CORES):
        shard = np.zeros((ROWS, D), np.float32)
        shard[:ROWS_REAL] = x[ci * ROWS_REAL : (ci + 1) * ROWS_REAL]
        in_maps.append({"x": shard, "ct": ct, "negn": negn, "ident": ident})
    res = bass_utils.run_bass_kernel_spmd(
        nc, in_maps, core_ids=list(range(N_CORES)), **spmd_kwargs
    )
    out = np.concatenate([r["out"][:ROWS_REAL] for r in res.results], axis=0)
    return out, res


def kernel(x, centroids):
    out, _ = run_sharded(x, centroids)
    return out


# revision 9
# speedup vs baseline: 2.7491x; 1.1637x over previous
"""Trainium2 Bass kernel: ClusterActivation (nearest-centroid routing +
per-row normalization + per-cluster activation).

Data-parallel over 8 NeuronCores: x sharded along rows (padded to a
multiple of 128*G), centroids replicated. Per core, rows are processed
in [128, 512] tiles, G tiles per group:

  - nearest centroid: PE transposes the x tile (f32 exact), ScalarE
    copies PSUM->SBUF, then f32 matmuls against centroid chunks
    accumulate [x @ c^T | row_sum] in PSUM (a 5th ones-column gives the
    row sum for free). A DVE scalar_tensor_tensor adds -|c_j|^2/2;
    argmax via reduce + is_equal gives one-hot masks.
  - stats: sum(x^2) via a DVE scalar_tensor_tensor with accum_out;
    mean from the PE ones-column; inv_std = rsqrt via Newton on DVE.
  - activations: two ScalarE passes from ONE LUT set
    (gelu_apprx_tanh_and_others):
      pass G: Gelu_apprx_tanh(x*sg + bg) covers gelu rows (scale α),
        relu rows (scale KF*α; gelu(KF z)/KF == relu(z) for large KF —
        LUT tail verified exact on HW), and sigmoid rows (scale 0,
        bias b* with gelu(b*)=0.5).
      pass T: Tanh(x*st + bt) covers tanh rows (a=1) and sigmoid rows
        (a=0.5; sigmoid(z) = 0.5 tanh(z/2) + 0.5).
    GpSimd computes c2 = tanh_out * a_t; DVE combines
    out = pg * W + c2 in one scalar_tensor_tensor.
  - per-row coefficient algebra is batched across the G tiles of a
    group ([128, G] ops), amortizing DVE instruction overhead.
"""

import numpy as np
from contextlib import ExitStack

import concourse.bacc as bacc
import concourse.mybir as mybir
from concourse.tile import TileContext
from concourse import bass_utils

F32 = mybir.dt.float32
AF = mybir.ActivationFunctionType
OP = mybir.AluOpType
AX = mybir.AxisListType

N_TOTAL, D, C = 100000, 512, 4
N_CORES = 8
P = 128
G = 14  # tiles per algebra/DMA window
ROWS = 12544  # padded rows per core: 98 tiles = 7 windows of 14
ROWS_REAL = N_TOTAL // N_CORES  # 12500
KCHUNKS = D // P  # 4
EPS = 1e-5
KF = 1024.0  # relu-via-gelu factor
INV_N = 1.0 / D
INV_NM1 = 1.0 / (D - 1.0)
BIAS_N = D * INV_NM1  # 512/511


def _solve_bstar():
    """b with gelu_tanh(b) == 0.5."""
    def g(z):
        return 0.5 * z * (1 + np.tanh(np.sqrt(2 / np.pi) * (z + 0.044715 * z**3)))

    lo, hi = 0.1, 1.5
    for _ in range(200):
        mid = 0.5 * (lo + hi)
        if g(mid) < 0.5:
            lo = mid
        else:
            hi = mid
    return float(0.5 * (lo + hi))


B_STAR = _solve_bstar()


def build_program(rows=ROWS, g_tiles=G):
    assert rows % (P * g_tiles) == 0
    n_groups = rows // (P * g_tiles)
    nc = bacc.Bacc("TRN2", target_bir_lowering=False, debug=False)
    x = nc.dram_tensor("x", [rows, D], F32, kind="ExternalInput").ap()
    # ct[d, k*(C+1) + j] = centroids[j, k*128+d] for j<C; ones for j=C
    ct = nc.dram_tensor("ct", [P, KCHUNKS * (C + 1)], F32, kind="ExternalInput").ap()
    # negn_rep[p, j] = -|c_j|^2/2 for j<C, 0 for j=C (replicated rows)
    negn = nc.dram_tensor("negn", [P, C + 1], F32, kind="ExternalInput").ap()
    ident = nc.dram_tensor("ident", [P, P], F32, kind="ExternalInput").ap()
    out = nc.dram_tensor("out", [rows, D], F32, kind="ExternalOutput").ap()

    with ExitStack() as ctx:
        tc = ctx.enter_context(TileContext(nc))
        cpool = ctx.enter_context(tc.tile_pool(name="const", bufs=1))
        xpool = ctx.enter_context(tc.tile_pool(name="xin", bufs=16))
        xtpool = ctx.enter_context(tc.tile_pool(name="xt", bufs=6))
        scpool = ctx.enter_context(tc.tile_pool(name="scratch", bufs=2))
        apool = ctx.enter_context(tc.tile_pool(name="acts", bufs=4))
        opool = ctx.enter_context(tc.tile_pool(name="outs", bufs=9))
        spool = ctx.enter_context(tc.tile_pool(name="small", bufs=2))
        ptpool = ctx.enter_context(tc.tile_pool(name="pxt", bufs=4, space="PSUM"))
        pspool = ctx.enter_context(tc.tile_pool(name="psc", bufs=4, space="PSUM"))

        ct_sb = cpool.tile([P, KCHUNKS, C + 1], F32)
        nc.sync.dma_start(ct_sb[:], ct.rearrange("d (k j) -> d k j", k=KCHUNKS))
        id_sb = cpool.tile([P, P], F32)
        nc.sync.dma_start(id_sb[:], ident)
        negn_sb = cpool.tile([P, C + 1], F32)
        nc.sync.dma_start(negn_sb[:], negn)

        xg = x.rearrange("(n g p) d -> n g p d", g=g_tiles, p=P)
        og = out.rearrange("(n g p) d -> n g p d", g=g_tiles, p=P)

        def part_a(n):
            """DMA + per-tile PE/stat work; returns state for algebra/part-B."""
            xt = []
            for h in range(g_tiles // 2):
                xsub = xpool.tile([P, 2, D], F32, tag="x", name=f"x_{n}_{h}")
                nc.sync.dma_start(
                    xsub[:], xg[n, 2 * h : 2 * h + 2].rearrange("g p d -> p g d")
                )
                xt.append(xsub)
            adj = spool.tile([P, g_tiles, C + 1], F32, tag="adj")
            ssq = spool.tile([P, g_tiles], F32, tag="ssq")
            smax = spool.tile([P, g_tiles], F32, tag="smax")
            masks = spool.tile([P, g_tiles, C], F32, tag="masks")
            return dict(xt=xt, adj=adj, ssq=ssq, smax=smax, masks=masks,
                        og=og, n=n)

        def part_a_tile(st, g):
            xt, adj, ssq, smax, masks = (
                st["xt"], st["adj"], st["ssq"], st["smax"], st["masks"]
            )
            xs = xt[g // 2][:, g % 2, :]
            pxt = ptpool.tile([P, KCHUNKS, P], F32, tag="pxt")
            for k in range(KCHUNKS):
                nc.tensor.transpose(
                    pxt[:, k, :], xs[:, k * P : (k + 1) * P], id_sb[:]
                )
            xtsb = xtpool.tile([P, KCHUNKS, P], F32, tag="xtsb")
            nc.scalar.copy(xtsb[:], pxt[:])
            psc = pspool.tile([P, C + 1], F32, tag="psc")
            for k in range(KCHUNKS):
                nc.tensor.matmul(
                    psc[:],
                    lhsT=xtsb[:, k, :],
                    rhs=ct_sb[:, k, :],
                    start=(k == 0),
                    stop=(k == KCHUNKS - 1),
                )
            nc.vector.scalar_tensor_tensor(
                adj[:, g, :], psc[:], 1.0, negn_sb[:], OP.mult, OP.add
            )
            sq = scpool.tile([P, D], F32, tag="sq")
            nc.vector.scalar_tensor_tensor(
                sq[:], xs, 1.0, xs, OP.bypass, OP.mult,
                accum_out=ssq[:, g : g + 1],
            )
            nc.vector.tensor_reduce(
                smax[:, g : g + 1], adj[:, g, 0:C], axis=AX.X, op=OP.max
            )
            nc.vector.tensor_scalar(
                masks[:, g, :], adj[:, g, 0:C], smax[:, g : g + 1], None,
                OP.is_equal,
            )

        def algebra(st):
            adj, ssq, masks = st["adj"], st["ssq"], st["masks"]
            m_r = masks[:, :, 0]
            m_t = masks[:, :, 1]
            m_g = masks[:, :, 2]
            m_s = masks[:, :, 3]
            mean = spool.tile([P, g_tiles], F32, tag="mean")
            nc.vector.tensor_scalar(mean[:], adj[:, :, C], INV_N, None, OP.mult)
            msq = spool.tile([P, g_tiles], F32, tag="msq")
            nc.vector.tensor_tensor(msq[:], mean[:], mean[:], OP.mult)
            u = spool.tile([P, g_tiles], F32, tag="u")
            nc.vector.tensor_scalar(u[:], msq[:], BIAS_N, -EPS, OP.mult, OP.add)
            vv = spool.tile([P, g_tiles], F32, tag="vv")
            nc.vector.scalar_tensor_tensor(
                vv[:], ssq[:], INV_NM1, u[:], OP.mult, OP.subtract
            )
            al = spool.tile([P, g_tiles], F32, tag="seed")
            nc.vector.tensor_scalar(al[:], vv[:], -0.5, 1.5, OP.mult, OP.add)
            for it in range(2):
                t2 = spool.tile([P, g_tiles], F32, tag=f"nw{it}a")
                nc.vector.tensor_tensor(t2[:], al[:], al[:], OP.mult)
                t3 = spool.tile([P, g_tiles], F32, tag=f"nw{it}b")
                nc.vector.tensor_tensor(t3[:], t2[:], vv[:], OP.mult)
                w = spool.tile([P, g_tiles], F32, tag=f"nw{it}c")
                nc.vector.tensor_scalar(w[:], t3[:], -0.5, 1.5, OP.mult, OP.add)
                al2 = spool.tile([P, g_tiles], F32, tag=f"nw{it}d")
                nc.vector.tensor_tensor(al2[:], al[:], w[:], OP.mult)
                al = al2
            alpha = al
            beta = spool.tile([P, g_tiles], F32, tag="beta")
            nc.vector.scalar_tensor_tensor(
                beta[:], mean[:], -1.0, alpha[:], OP.mult, OP.mult
            )
            a_t = spool.tile([P, g_tiles], F32, tag="a_t")
            nc.vector.scalar_tensor_tensor(a_t[:], m_s, 0.5, m_t, OP.mult, OP.add)
            s_t = spool.tile([P, g_tiles], F32, tag="s_t")
            nc.vector.tensor_tensor(s_t[:], a_t[:], alpha[:], OP.mult)
            b_t = spool.tile([P, g_tiles], F32, tag="b_t")
            nc.vector.tensor_tensor(b_t[:], a_t[:], beta[:], OP.mult)
            u2 = spool.tile([P, g_tiles], F32, tag="u2")
            nc.vector.scalar_tensor_tensor(u2[:], m_r, KF, m_g, OP.mult, OP.add)
            s_g = spool.tile([P, g_tiles], F32, tag="s_g")
            nc.vector.tensor_tensor(s_g[:], u2[:], alpha[:], OP.mult)
            t4 = spool.tile([P, g_tiles], F32, tag="t4")
            nc.vector.tensor_tensor(t4[:], u2[:], beta[:], OP.mult)
            b_g = spool.tile([P, g_tiles], F32, tag="b_g")
            nc.vector.scalar_tensor_tensor(b_g[:], m_s, B_STAR, t4[:], OP.mult, OP.add)
            wv3 = spool.tile([P, g_tiles], F32, tag="wv3")
            nc.vector.scalar_tensor_tensor(wv3[:], m_r, 1.0 / KF, m_s, OP.mult, OP.add)
            wfin = spool.tile([P, g_tiles], F32, tag="wfin")
            nc.vector.tensor_tensor(wfin[:], wv3[:], m_g, OP.add)
            st["a_t"] = a_t
            st["s_t"] = s_t
            st["b_t"] = b_t
            st["s_g"] = s_g
            st["b_g"] = b_g
            st["wfin"] = wfin
            st["og_t"] = [
                opool.tile([P, 2, D], F32, tag="ot", name=f"og_t{h}")
                for h in range(g_tiles // 2)
            ]

        def part_b_tile(st, g):
            xs = st["xt"][g // 2][:, g % 2, :]
            pg = apool.tile([P, D], F32, tag="pg")
            nc.scalar.activation(
                pg[:], xs, AF.Gelu_apprx_tanh,
                bias=st["b_g"][:, g : g + 1], scale=st["s_g"][:, g : g + 1],
            )
            th = apool.tile([P, D], F32, tag="th")
            nc.scalar.activation(
                th[:], xs, AF.Tanh,
                bias=st["b_t"][:, g : g + 1], scale=st["s_t"][:, g : g + 1],
            )
            c2 = apool.tile([P, D], F32, tag="c2")
            nc.vector.tensor_scalar(
                c2[:], th[:], st["a_t"][:, g : g + 1], None, OP.mult
            )
            nc.vector.scalar_tensor_tensor(
                st["og_t"][g // 2][:, g % 2, :], pg[:],
                st["wfin"][:, g : g + 1], c2[:],
                OP.mult, OP.add,
            )
            if g % 2 == 1:
                nc.sync.dma_start(
                    st["og"][st["n"], g - 1 : g + 1].rearrange("g p d -> p g d"),
                    st["og_t"][g // 2][:],
                )

        # software-pipelined emission: window n part-A interleaved with
        # window n-1 part-B so every engine's queue alternates roles.
        prev = None
        for n in range(n_groups):
            st = part_a(n)
            for g in range(g_tiles):
                part_a_tile(st, g)
                if prev is not None:
                    part_b_tile(prev, g)
            algebra(st)
            prev = st
        for g in range(g_tiles):
            part_b_tile(prev, g)

    nc.compile()
    return nc


_nc_cache = {}


def _get_nc(rows):
    if rows not in _nc_cache:
        _nc_cache[rows] = build_program(rows)
    return _nc_cache[rows]


def make_const_inputs(centroids):
    c = np.asarray(centroids, dtype=np.float32)
    ct = np.zeros((P, KCHUNKS * (C + 1)), np.float32)
    for k in range(KCHUNKS):
        ct[:, k * (C + 1) : k * (C + 1) + C] = c[:, k * P : (k + 1) * P].T
        ct[:, k * (C + 1) + C] = 1.0
    negn = np.zeros((P, C + 1), np.float32)
    negn[:, :C] = (-0.5 * np.sum(c.astype(np.float64) ** 2, axis=1)).astype(np.float32)
    ident = np.eye(P, dtype=np.float32)
    return ct, negn, ident


def run_sharded(x, centroids, **spmd_kwargs):
    x = np.ascontiguousarray(np.asarray(x), dtype=np.float32)
    assert x.shape == (N_TOTAL, D)
    nc = _get_nc(ROWS)
    ct, negn, ident = make_const_inputs(centroids)
    in_maps = []
    for ci in range(N_CORES):
        shard = np.zeros((ROWS, D), np.float32)
        shard[:ROWS_REAL] = x[ci * ROWS_REAL : (ci + 1) * ROWS_REAL]
        in_maps.append({"x": shard, "ct": ct, "negn": negn, "ident": ident})
    res = bass_utils.run_bass_kernel_spmd(
        nc, in_maps, core_ids=list(range(N_CORES)), **spmd_kwargs
    )
    out = np.concatenate([r["out"][:ROWS_REAL] for r in res.results], axis=0)
    return out, res


def kernel(x, centroids):
    out, _ = run_sharded(x, centroids)
    return out
